# revision 1
# baseline (speedup 1.0000x reference)
"""DeepSeek-V3 token-choice top-k router on 8 Trainium2 NeuronCores.

Strategy (per core, data-parallel over tokens; 1024 tokens/core):
  - x shard [1024, 7168] fp32 streamed from HBM in 8 token-tiles of [128, 7168].
  - Gate weight split on host into fp32r hi/lo pair (exact: hi + lo == w in
    fp32) and packed to [128, 56*256] d-major chunks, replicated per core.
  - PE: per 128-token tile, transpose x chunks ([128t,128d] -> [128d,128t]);
    ACT casts the PSUM transpose to fp32r (hi, round-to-nearest); DVE computes
    lo = x_T - hi (exact Sterbenz subtract, cast fp32r keeps all useful bits).
    Then 3 accumulating fp32r matmuls per chunk (hi@w_hi + hi@w_lo + lo@w_hi;
    the dropped lo@w_lo term is ~2^-26 relative) -> exact-fp32-grade logits
    [128 tokens, 256 experts] in PSUM at 1 cycle/row instead of fp32's 4.
  - ACT: sigmoid(logits) PSUM->SBUF.
  - DVE: hardware top-8 (`max`/`max_index`) for group top-2 sums, top-4 group
    threshold, masked top-8; normalization.
  - GPSIMD: bias add, group masking, and the one-hot weight gathers
    (scalar_tensor_tensor with accumulate), keeping DVE under the PE span.
"""

import numpy as np

N = 8192
D = 7168
E = 256
G = 8
EPG = E // G  # 32
TOPK_GROUP = 4
TOP_K = 8
SCALING = 2.5
N_CORES = 8
NPC = N // N_CORES  # 1024 tokens per core
P = 128
KC = D // P  # 56 contraction chunks
TT = NPC // P  # 8 token tiles per core
KB = 4  # k-chunks per transpose batch (one PSUM bank)
NB = KC // KB  # 14 batches

_CACHE = {}


def build_program(mode="f32r_3pass"):
    import concourse.bacc as bacc
    import concourse.mybir as mybir
    from concourse import tile, masks

    nc = bacc.Bacc(
        "TRN2",
        target_bir_lowering=False,
        debug=False,
        enable_asserts=True,
        num_devices=N_CORES,
    )
    f32 = mybir.dt.float32
    f32r = mybir.dt.float32r
    i32 = mybir.dt.int32
    u32 = mybir.dt.uint32
    AF = mybir.ActivationFunctionType
    OP = mybir.AluOpType
    AX = mybir.AxisListType

    three_pass = mode == "f32r_3pass"
    gdt = f32r if three_pass else f32

    x_d = nc.dram_tensor("x", [NPC, D], f32, kind="ExternalInput").ap()
    if three_pass:
        gwh_d = nc.dram_tensor("gw2", [P, KC * 2 * E], f32r, kind="ExternalInput").ap()
    else:
        gwh_d = nc.dram_tensor("gwh", [P, KC * E], gdt, kind="ExternalInput").ap()
    bias_d = nc.dram_tensor("bias", [1, E], f32, kind="ExternalInput").ap()
    idx_d = nc.dram_tensor("idx", [NPC, TOP_K], i32, kind="ExternalOutput").ap()
    w_d = nc.dram_tensor("w", [NPC, TOP_K], f32, kind="ExternalOutput").ap()

    with tile.TileContext(nc) as tc:
        with (
            tc.tile_pool(name="const", bufs=1) as const_pool,
            tc.tile_pool(name="gw", bufs=1) as gw_pool,
            tc.tile_pool(name="x", bufs=2) as x_pool,
            tc.tile_pool(name="xt", bufs=4) as xt_pool,
            tc.tile_pool(name="ptr", bufs=3, space="PSUM") as ptr_pool,
            tc.tile_pool(name="plog", bufs=2, space="PSUM") as plog_pool,
            tc.tile_pool(name="work", bufs=2) as work_pool,
            tc.tile_pool(name="outs", bufs=2) as out_pool,
        ):
            # ---- tiny bias DMA first, then first x tile, then gw quarters ----
            bias_sb = const_pool.tile([1, E], f32, name="biassb")
            nc.sync.dma_start(bias_sb[:], bias_d[:])
            x_tiles = {}
            x_tiles[0] = x_pool.tile([P, D], f32, tag="xtile", name="xtile0")
            nc.sync.dma_start(x_tiles[0][:], x_d[0:P, :])

            # ---- gate weight DMA in quarters; x1 issued between q1 and q2
            if three_pass:
                gwh_sb = gw_pool.tile([P, KC * 2 * E], f32r, name="gw2sb")
                q = KC * 2 * E // 4
                nc.sync.dma_start(gwh_sb[:, 0 * q : 1 * q], gwh_d[:, 0 * q : 1 * q])
                nc.sync.dma_start(gwh_sb[:, 1 * q : 2 * q], gwh_d[:, 1 * q : 2 * q])
                x_tiles[1] = x_pool.tile([P, D], f32, tag="xtile", name="xtile1")
                nc.sync.dma_start(x_tiles[1][:], x_d[P : 2 * P, :])
                nc.sync.dma_start(gwh_sb[:, 2 * q : 3 * q], gwh_d[:, 2 * q : 3 * q])
                nc.sync.dma_start(gwh_sb[:, 3 * q : 4 * q], gwh_d[:, 3 * q : 4 * q])
                gwh_v = gwh_sb[:].rearrange("p (k e) -> p k e", k=KC)
            else:
                gwh_sb = gw_pool.tile([P, KC * E], gdt, name="gwhsb")
                nc.sync.dma_start(gwh_sb[:], gwh_d[:])
                gwh_v = gwh_sb[:].rearrange("p (k e) -> p k e", k=KC)

            # ---- constants ----
            ident = const_pool.tile([P, P], f32)
            masks.make_identity(nc, ident[:])
            iota_i = const_pool.tile([P, E], i32)
            nc.gpsimd.iota(iota_i[:], pattern=[[1, E]], base=0, channel_multiplier=0)
            iota_f = const_pool.tile([P, E], f32)
            nc.vector.tensor_copy(iota_f[:], iota_i[:])
            bias_rep = const_pool.tile([P, E], f32)
            nc.gpsimd.partition_broadcast(bias_rep[:], bias_sb[0:1, :])

            for t in range(TT):
                if t not in x_tiles:
                    x_tiles[t] = x_pool.tile([P, D], f32, tag="xtile", name=f"xtile{t}")
                    nc.sync.dma_start(x_tiles[t][:], x_d[t * P : (t + 1) * P, :])
                if t + 1 < TT and (t + 1) not in x_tiles:
                    x_tiles[t + 1] = x_pool.tile([P, D], f32, tag="xtile", name=f"xtile{t+1}")
                    nc.sync.dma_start(
                        x_tiles[t + 1][:], x_d[(t + 1) * P : (t + 2) * P, :]
                    )
                x_tile = x_tiles[t]

                if three_pass:
                    plog = plog_pool.tile([P, 2 * E], f32, tag="plog")
                else:
                    plog = plog_pool.tile([P, E], f32, tag="plog")
                for b in range(NB):
                    ptr = ptr_pool.tile([P, KB * P], f32, tag="ptr")
                    for j in range(KB):
                        k = b * KB + j
                        nc.tensor.matmul(
                            ptr[:, j * P : (j + 1) * P],
                            x_tile[:, k * P : (k + 1) * P],
                            ident[:],
                            is_transpose=True,
                        )
                    if three_pass:
                        hi_sb = xt_pool.tile([P, KB * P], f32r, tag="hi")
                        nc.scalar.copy(hi_sb[:], ptr[:])
                        lo_sb = xt_pool.tile([P, KB * P], f32r, tag="lo")
                        nc.vector.scalar_tensor_tensor(
                            lo_sb[:], ptr[:], 0.0, hi_sb[:].bitcast(f32),
                            op0=OP.add, op1=OP.subtract,
                        )
                        for j in range(KB):
                            k = b * KB + j
                            sl = slice(j * P, (j + 1) * P)
                            nc.tensor.matmul(
                                plog[:], hi_sb[:, sl], gwh_v[:, k, :],
                                start=(k == 0), stop=False,
                            )
                            nc.tensor.matmul(
                                plog[:, 0:E], lo_sb[:, sl], gwh_v[:, k, 0:E],
                                start=False, stop=(k == KC - 1),
                            )
                    else:
                        xt_sb = xt_pool.tile([P, KB * P], f32, tag="hi")
                        nc.scalar.copy(xt_sb[:], ptr[:])
                        for j in range(KB):
                            k = b * KB + j
                            nc.tensor.matmul(
                                plog[:],
                                xt_sb[:, j * P : (j + 1) * P],
                                gwh_v[:, k, :],
                                start=(k == 0), stop=(k == KC - 1),
                            )

                # ---- routing for this token tile ----
                scores = work_pool.tile([P, E], f32, tag="scores")
                if three_pass:
                    half2 = work_pool.tile([P, E], f32, tag="half2")
                    nc.scalar.copy(half2[:], plog[:, E : 2 * E])
                    lsum = work_pool.tile([P, E], f32, tag="lsum")
                    nc.vector.tensor_tensor(
                        lsum[:], plog[:, 0:E], half2[:], op=OP.add
                    )
                    nc.scalar.activation(scores[:], lsum[:], AF.Sigmoid)
                else:
                    nc.scalar.activation(scores[:], plog[:], AF.Sigmoid)

                sfc = work_pool.tile([P, E], f32, tag="sfc")
                nc.gpsimd.tensor_tensor(sfc[:], scores[:], bias_rep[:], op=OP.add)

                # per-group top-8 (need top-2 of each group of 32)
                gtops = work_pool.tile([P, G * 8], f32, tag="gtops")
                for g in range(G):
                    nc.vector.max(
                        gtops[:, g * 8 : (g + 1) * 8],
                        sfc[:, g * EPG : (g + 1) * EPG],
                    )
                gv = gtops[:].rearrange("p (g k) -> p g k", g=G)
                gs = work_pool.tile([P, G], f32, tag="gs")
                nc.vector.tensor_tensor(gs[:], gv[:, :, 0], gv[:, :, 1], op=OP.add)

                # top-4 groups -> mask
                gtop8 = work_pool.tile([P, 8], f32, tag="gtop8")
                nc.vector.max(gtop8[:], gs[:])
                gmask = work_pool.tile([P, G], f32, tag="gmask")
                nc.vector.tensor_scalar(
                    gmask[:], gs[:], gtop8[:, TOPK_GROUP - 1 : TOPK_GROUP], None,
                    op0=OP.is_ge,
                )

                # masked scores
                tmp = work_pool.tile([P, E], f32, tag="tmp")
                for g in range(G):
                    nc.vector.tensor_scalar(
                        tmp[:, g * EPG : (g + 1) * EPG],
                        sfc[:, g * EPG : (g + 1) * EPG],
                        gmask[:, g : g + 1],
                        None,
                        op0=OP.mult,
                    )

                # top-8 values + indices
                vals = work_pool.tile([P, TOP_K], f32, tag="vals")
                nc.vector.max(vals[:], tmp[:])
                idxu = work_pool.tile([P, TOP_K], u32, tag="idxu")
                nc.vector.max_index(idxu[:], vals[:], tmp[:])
                idxf = work_pool.tile([P, TOP_K], f32, tag="idxf")
                nc.vector.tensor_copy(idxf[:], idxu[:])

                # gather raw sigmoid scores at the selected indices (GPSIMD)
                w8 = out_pool.tile([P, TOP_K], f32, tag="w8")
                scratch = work_pool.tile([P, E], f32, tag="scratch")
                for j in range(TOP_K):
                    nc.vector.scalar_tensor_tensor(
                        scratch[:],
                        iota_f[:],
                        idxf[:, j : j + 1],
                        scores[:],
                        op0=OP.is_equal,
                        op1=OP.mult,
                        accum_out=w8[:, j : j + 1],
                    )

                # normalize + scale
                wsum = work_pool.tile([P, 1], f32, tag="wsum")
                nc.vector.reduce_sum(wsum[:], w8[:], axis=AX.X)
                wse = work_pool.tile([P, 1], f32, tag="wse")
                nc.vector.tensor_scalar(wse[:], wsum[:], 1e-20, None, op0=OP.add)
                wrec = work_pool.tile([P, 1], f32, tag="wrec")
                nc.vector.reciprocal(wrec[:], wse[:])
                w_out = out_pool.tile([P, TOP_K], f32, tag="wout")
                nc.vector.tensor_scalar(
                    w_out[:], w8[:], wrec[:, 0:1], float(SCALING),
                    op0=OP.mult, op1=OP.mult,
                )
                idx_out = out_pool.tile([P, TOP_K], i32, tag="idxout")
                nc.vector.tensor_copy(idx_out[:], idxu[:])

                nc.sync.dma_start(idx_d[t * P : (t + 1) * P, :], idx_out[:])
                nc.sync.dma_start(w_d[t * P : (t + 1) * P, :], w_out[:])

    nc.compile()
    return nc


def _get_nc(**kw):
    key = tuple(sorted(kw.items()))
    if key not in _CACHE:
        _CACHE[key] = build_program(**kw)
    return _CACHE[key]


def _fp32r_round(a):
    # round-to-nearest fp32 -> fp32r (12-bit mantissa), bit-exact with HW cast
    bits = np.ascontiguousarray(a).view(np.uint32)
    keep = np.uint32(0xFFFFF000)
    rounded = (bits + np.uint32(0x800)) & keep  # round-half-up into kept bits
    # correct round-to-nearest-even on the halfway case
    half = (bits & np.uint32(0xFFF)) == np.uint32(0x800)
    even = ((bits >> np.uint32(12)) & np.uint32(1)) == 0
    rounded = np.where(half & even, bits & keep, rounded)
    return rounded.view(np.float32).reshape(a.shape)


def _pack(a2d):
    # [D, E] -> [P, KC*E]: partition p holds rows k*128+p
    return np.ascontiguousarray(
        a2d.reshape(KC, P, E).transpose(1, 0, 2)
    ).reshape(P, KC * E)


def _run(x, gate_w, bias, trace=False, **build_kw):
    from concourse.bass_utils import run_bass_kernel_spmd

    x = np.ascontiguousarray(np.asarray(x, dtype=np.float32))
    gate_w = np.ascontiguousarray(np.asarray(gate_w, dtype=np.float32))
    bias = np.ascontiguousarray(np.asarray(bias, dtype=np.float32))
    nc = _get_nc(**build_kw)
    mode = build_kw.get("mode", "f32r_3pass")
    gwt = np.ascontiguousarray(gate_w.T)  # [D, E]
    bias2d = bias.reshape(1, E)
    if mode == "f32r_3pass":
        gw_hi = _fp32r_round(gwt)
        gw_lo = _fp32r_round(gwt - gw_hi)
        ph = _pack(gw_hi).reshape(P, KC, E)
        pl = _pack(gw_lo).reshape(P, KC, E)
        gw2 = np.concatenate([ph, pl], axis=2).reshape(P, KC * 2 * E)
        maps = {"gw2": np.ascontiguousarray(gw2), "bias": bias2d}
    else:
        maps = {"gwh": _pack(gwt), "bias": bias2d}
    in_maps = [
        {"x": x[c * NPC : (c + 1) * NPC], **maps} for c in range(N_CORES)
    ]
    res = run_bass_kernel_spmd(nc, in_maps, core_ids=list(range(N_CORES)), trace=trace)
    idx = np.concatenate([res.results[c]["idx"] for c in range(N_CORES)], axis=0)
    w = np.concatenate([res.results[c]["w"] for c in range(N_CORES)], axis=0)
    return (idx.astype(np.int32), w.astype(np.float32)), res


def kernel(x, gate_w, bias):
    (idx, w), _ = _run(x, gate_w, bias)
    return idx, w



# revision 2
# speedup vs baseline: 1.5111x; 1.5111x over previous
"""DeepSeek-V3 token-choice top-k router on 8 Trainium2 NeuronCores.

Strategy (per core, data-parallel over tokens; 1024 tokens/core):
  - Host: x and gate_w.T are scaled by 4096 (exact power of 2) and split into
    fp16 hi/lo pairs (hi = rn(a), lo = rn(a - hi); 22+ mantissa bits combined,
    scaling keeps everything out of the fp16 subnormal range). x is also
    pre-transposed to d-major [128d, token] chunk layout so the device does
    ZERO transposes and zero precision-split work.
  - Device per 128-token tile: 56 contraction chunks x 3 accumulating fp16
    matmuls (xh@wh + xh@wl + xl@wh; dropped xl@wl term is ~2^-22 relative)
    into a [128, 256] PSUM logits tile. fp16 streams the PE at 1 col/cycle
    (same as f32r) but halves gate DMA and gets 2x faster LDWEIGHTS via FWL.
  - ACT: sigmoid(logits * 2^-24) PSUM->SBUF (scale undone for free).
  - DVE: hardware top-8 (`max`/`max_index`) for group top-2 sums, top-4 group
    threshold, masked top-8, one-hot gathers of the raw scores, normalization.
  - GPSIMD: bias add (keeps DVE under the PE span).
"""

import numpy as np

N = 8192
D = 7168
E = 256
G = 8
EPG = E // G  # 32
TOPK_GROUP = 4
TOP_K = 8
SCALING = 2.5
N_CORES = 8
NPC = N // N_CORES  # 1024 tokens per core
P = 128
KC = D // P  # 56 contraction chunks
TT = NPC // P  # 8 token tiles per core
GPC = 7  # gate-weight chunks per DMA piece
NGP = KC // GPC  # 8 pieces
SX = 4096.0  # x scale (2^12)
SW = 4096.0  # w scale (2^12)

_CACHE = {}


def build_program():
    import concourse.bacc as bacc
    import concourse.mybir as mybir
    from concourse import tile

    nc = bacc.Bacc(
        "TRN2",
        target_bir_lowering=False,
        debug=False,
        enable_asserts=True,
        num_devices=N_CORES,
    )
    f16 = mybir.dt.float16
    f32 = mybir.dt.float32
    i32 = mybir.dt.int32
    u32 = mybir.dt.uint32
    AF = mybir.ActivationFunctionType
    OP = mybir.AluOpType
    AX = mybir.AxisListType

    xh_d = nc.dram_tensor("xh", [P, TT * KC * P], f16, kind="ExternalInput").ap()
    xl_d = nc.dram_tensor("xl", [P, TT * KC * P], f16, kind="ExternalInput").ap()
    gw_d = nc.dram_tensor("gw2", [P, KC * 2 * E], f16, kind="ExternalInput").ap()
    bias_d = nc.dram_tensor("bias", [1, E], f32, kind="ExternalInput").ap()
    idx_d = nc.dram_tensor("idx", [NPC, TOP_K], i32, kind="ExternalOutput").ap()
    w_d = nc.dram_tensor("w", [NPC, TOP_K], f32, kind="ExternalOutput").ap()

    with tile.TileContext(nc) as tc:
        with (
            tc.tile_pool(name="const", bufs=1) as const_pool,
            tc.tile_pool(name="gw", bufs=1) as gw_pool,
            tc.tile_pool(name="xh", bufs=2) as xh_pool,
            tc.tile_pool(name="xl", bufs=2) as xl_pool,
            tc.tile_pool(name="plog", bufs=2, space="PSUM") as plog_pool,
            tc.tile_pool(name="work", bufs=2) as work_pool,
            tc.tile_pool(name="outs", bufs=2) as out_pool,
        ):
            # ---- DMA order: bias, first x tile, gw piece 0, second x tile,
            # remaining gw pieces (chunk-major so tile-0 matmuls can start
            # before the full gate weight has landed).
            bias_sb = const_pool.tile([1, E], f32, name="biassb")
            nc.sync.dma_start(bias_sb[:], bias_d[:])
            xh_tiles, xl_tiles = {}, {}

            def fetch_x(t):
                xh_tiles[t] = xh_pool.tile([P, KC * P], f16, tag="xh", name=f"xh{t}")
                xl_tiles[t] = xl_pool.tile([P, KC * P], f16, tag="xl", name=f"xl{t}")
                sl = slice(t * KC * P, (t + 1) * KC * P)
                nc.sync.dma_start(xh_tiles[t][:], xh_d[:, sl])
                nc.sync.dma_start(xl_tiles[t][:], xl_d[:, sl])

            fetch_x(0)
            gw_sb = []
            q = GPC * 2 * E
            for i in range(NGP):
                gw_sb.append(gw_pool.tile([P, q], f16, name=f"gw{i}"))
            nc.sync.dma_start(gw_sb[0][:], gw_d[:, 0:q])
            fetch_x(1)
            for i in range(1, NGP):
                nc.sync.dma_start(gw_sb[i][:], gw_d[:, i * q : (i + 1) * q])
            gw_v = [g[:].rearrange("p (k e) -> p k e", k=GPC) for g in gw_sb]

            # ---- constants ----
            iota_i = const_pool.tile([P, E], i32)
            nc.gpsimd.iota(iota_i[:], pattern=[[1, E]], base=0, channel_multiplier=0)
            iota_f = const_pool.tile([P, E], f32)
            nc.vector.tensor_copy(iota_f[:], iota_i[:])
            bias_rep = const_pool.tile([P, E], f32)
            nc.gpsimd.partition_broadcast(bias_rep[:], bias_sb[0:1, :])

            for t in range(TT):
                if t + 1 < TT and (t + 1) not in xh_tiles:
                    fetch_x(t + 1)
                xh_t, xl_t = xh_tiles[t], xl_tiles[t]

                plog = plog_pool.tile([P, E], f32, tag="plog")
                for k in range(KC):
                    pc, sk = divmod(k, GPC)
                    wh = gw_v[pc][:, sk, 0:E]
                    wl = gw_v[pc][:, sk, E : 2 * E]
                    xh_k = xh_t[:, k * P : (k + 1) * P]
                    xl_k = xl_t[:, k * P : (k + 1) * P]
                    nc.tensor.matmul(plog[:], xh_k, wh, start=(k == 0), stop=False)
                    nc.tensor.matmul(plog[:], xh_k, wl, start=False, stop=False)
                    nc.tensor.matmul(
                        plog[:], xl_k, wh, start=False, stop=(k == KC - 1)
                    )

                # ---- routing for this token tile ----
                scores = work_pool.tile([P, E], f32, tag="scores")
                nc.scalar.activation(
                    scores[:], plog[:], AF.Sigmoid, scale=1.0 / (SX * SW)
                )

                sfc = work_pool.tile([P, E], f32, tag="sfc")
                nc.gpsimd.tensor_tensor(sfc[:], scores[:], bias_rep[:], op=OP.add)

                # per-group top-8 (need top-2 of each group of 32)
                gtops = work_pool.tile([P, G * 8], f32, tag="gtops")
                for g in range(G):
                    nc.vector.max(
                        gtops[:, g * 8 : (g + 1) * 8],
                        sfc[:, g * EPG : (g + 1) * EPG],
                    )
                gv = gtops[:].rearrange("p (g k) -> p g k", g=G)
                gs = work_pool.tile([P, G], f32, tag="gs")
                nc.vector.tensor_tensor(gs[:], gv[:, :, 0], gv[:, :, 1], op=OP.add)

                # top-4 groups -> mask
                gtop8 = work_pool.tile([P, 8], f32, tag="gtop8")
                nc.vector.max(gtop8[:], gs[:])
                gmask = work_pool.tile([P, G], f32, tag="gmask")
                nc.vector.tensor_scalar(
                    gmask[:], gs[:], gtop8[:, TOPK_GROUP - 1 : TOPK_GROUP], None,
                    op0=OP.is_ge,
                )

                # masked scores
                tmp = work_pool.tile([P, E], f32, tag="tmp")
                for g in range(G):
                    nc.vector.tensor_scalar(
                        tmp[:, g * EPG : (g + 1) * EPG],
                        sfc[:, g * EPG : (g + 1) * EPG],
                        gmask[:, g : g + 1],
                        None,
                        op0=OP.mult,
                    )

                # top-8 values + indices
                vals = work_pool.tile([P, TOP_K], f32, tag="vals")
                nc.vector.max(vals[:], tmp[:])
                idxu = work_pool.tile([P, TOP_K], u32, tag="idxu")
                nc.vector.max_index(idxu[:], vals[:], tmp[:])
                idxf = work_pool.tile([P, TOP_K], f32, tag="idxf")
                nc.vector.tensor_copy(idxf[:], idxu[:])

                # gather raw sigmoid scores at the selected indices
                w8 = out_pool.tile([P, TOP_K], f32, tag="w8")
                scratch = work_pool.tile([P, E], f32, tag="scratch")
                for j in range(TOP_K):
                    nc.vector.scalar_tensor_tensor(
                        scratch[:],
                        iota_f[:],
                        idxf[:, j : j + 1],
                        scores[:],
                        op0=OP.is_equal,
                        op1=OP.mult,
                        accum_out=w8[:, j : j + 1],
                    )

                # normalize + scale
                wsum = work_pool.tile([P, 1], f32, tag="wsum")
                nc.vector.reduce_sum(wsum[:], w8[:], axis=AX.X)
                wse = work_pool.tile([P, 1], f32, tag="wse")
                nc.vector.tensor_scalar(wse[:], wsum[:], 1e-20, None, op0=OP.add)
                wrec = work_pool.tile([P, 1], f32, tag="wrec")
                nc.vector.reciprocal(wrec[:], wse[:])
                w_out = out_pool.tile([P, TOP_K], f32, tag="wout")
                nc.vector.tensor_scalar(
                    w_out[:], w8[:], wrec[:, 0:1], float(SCALING),
                    op0=OP.mult, op1=OP.mult,
                )
                idx_out = out_pool.tile([P, TOP_K], i32, tag="idxout")
                nc.vector.tensor_copy(idx_out[:], idxu[:])

                nc.sync.dma_start(idx_d[t * P : (t + 1) * P, :], idx_out[:])
                nc.sync.dma_start(w_d[t * P : (t + 1) * P, :], w_out[:])

    nc.compile()
    return nc


def _get_nc(**kw):
    key = tuple(sorted(kw.items()))
    if key not in _CACHE:
        _CACHE[key] = build_program(**kw)
    return _CACHE[key]


def _pack_x(a):
    # [8192, 7168] -> [8 cores, 128, TT*KC*128]; per core, partition p holds
    # d = k*128+p, free axis is (token tile t, chunk k, token j) -> x.T chunks.
    return np.ascontiguousarray(
        a.reshape(N_CORES, TT, P, KC, P).transpose(0, 4, 1, 3, 2)
    ).reshape(N_CORES, P, TT * KC * P)


def _prep_inputs(x, gate_w, bias):
    xs = x * np.float32(SX)
    xh = xs.astype(np.float16)
    xl = (xs - xh.astype(np.float32)).astype(np.float16)
    xhp = _pack_x(xh)
    xlp = _pack_x(xl)

    ws = np.ascontiguousarray(gate_w.T) * np.float32(SW)  # [D, E]
    wh = ws.astype(np.float16)
    wl = (ws - wh.astype(np.float32)).astype(np.float16)
    gw2 = np.concatenate(
        [wh.reshape(KC, P, E), wl.reshape(KC, P, E)], axis=2
    )  # [KC, P, 2E]
    gw2 = np.ascontiguousarray(gw2.transpose(1, 0, 2)).reshape(P, KC * 2 * E)
    bias2d = np.ascontiguousarray(bias.reshape(1, E))
    return xhp, xlp, gw2, bias2d


def _run(x, gate_w, bias, trace=False, **build_kw):
    from concourse.bass_utils import run_bass_kernel_spmd

    x = np.ascontiguousarray(np.asarray(x, dtype=np.float32))
    gate_w = np.ascontiguousarray(np.asarray(gate_w, dtype=np.float32))
    bias = np.ascontiguousarray(np.asarray(bias, dtype=np.float32))
    nc = _get_nc(**build_kw)
    xhp, xlp, gw2, bias2d = _prep_inputs(x, gate_w, bias)
    in_maps = [
        {"xh": xhp[c], "xl": xlp[c], "gw2": gw2, "bias": bias2d}
        for c in range(N_CORES)
    ]
    res = run_bass_kernel_spmd(nc, in_maps, core_ids=list(range(N_CORES)), trace=trace)
    idx = np.concatenate([res.results[c]["idx"] for c in range(N_CORES)], axis=0)
    w = np.concatenate([res.results[c]["w"] for c in range(N_CORES)], axis=0)
    return (idx.astype(np.int32), w.astype(np.float32)), res


def kernel(x, gate_w, bias):
    (idx, w), _ = _run(x, gate_w, bias)
    return idx, w


# revision 3
# speedup vs baseline: 1.6536x; 1.0943x over previous
"""DeepSeek-V3 token-choice top-k router on 8 Trainium2 NeuronCores.

Strategy (per core, data-parallel over tokens; 1024 tokens/core):
  - Host: x and gate_w.T are scaled by 4096 (exact power of 2) and split into
    fp16 hi/lo pairs (hi = rn(a), lo = rn(a - hi); 22+ mantissa bits combined,
    scaling keeps everything out of the fp16 subnormal range). x is also
    pre-transposed to d-major [128d, token] chunk layout so the device does
    ZERO transposes and zero precision-split work.
  - Device per 128-token tile: 56 contraction chunks x 3 accumulating fp16
    matmuls (xh@wh + xh@wl + xl@wh; dropped xl@wl term is ~2^-22 relative)
    into a [128, 256] PSUM logits tile. fp16 streams the PE at 1 col/cycle
    (same as f32r) but halves gate DMA and speeds LDWEIGHTS (FWL-eligible).
  - All input DMA goes through the single SP HWDGE ring in a hand-interleaved
    order (x0/x1 in pieces between gw pieces) so tile-0/1 matmuls start ~6us
    in and stay just-in-time fed; output DMA rides the ACT ring.
  - ~70 tiny warmup matmuls run during the initial DMA wait to flip the PE
    HAM clock gate to 2.4 GHz before real work arrives.
  - ACT: sigmoid(logits * 2^-24) PSUM->SBUF (scale undone for free).
  - DVE: bias add, hardware top-8 (`max`/`max_index`) for group top-2 sums,
    top-4 group threshold, masked top-8, one-hot score gathers, normalization.
"""

import numpy as np

N = 8192
D = 7168
E = 256
G = 8
EPG = E // G  # 32
TOPK_GROUP = 4
TOP_K = 8
SCALING = 2.5
N_CORES = 8
NPC = N // N_CORES  # 1024 tokens per core
P = 128
KC = D // P  # 56 contraction chunks
TT = NPC // P  # 8 token tiles per core
GPC = 7  # gate-weight chunks per DMA piece
NGP = KC // GPC  # 8 pieces
SX = 4096.0  # x scale (2^12)
SW = 4096.0  # w scale (2^12)
X0P = 8  # tile-0 x DMA pieces (7 chunks each)
X1P = 4  # tile-1 x DMA pieces (14 chunks each)
WARMUP_MM = 72

_CACHE = {}


def build_program():
    import concourse.bacc as bacc
    import concourse.mybir as mybir
    from concourse import tile

    nc = bacc.Bacc(
        "TRN2",
        target_bir_lowering=False,
        debug=False,
        enable_asserts=True,
        num_devices=N_CORES,
    )
    f16 = mybir.dt.float16
    f32 = mybir.dt.float32
    i32 = mybir.dt.int32
    u32 = mybir.dt.uint32
    AF = mybir.ActivationFunctionType
    OP = mybir.AluOpType
    AX = mybir.AxisListType

    xh_d = nc.dram_tensor("xh", [P, TT * KC * P], f16, kind="ExternalInput").ap()
    xl_d = nc.dram_tensor("xl", [P, TT * KC * P], f16, kind="ExternalInput").ap()
    gw_d = nc.dram_tensor("gw2", [P, KC * 2 * E], f16, kind="ExternalInput").ap()
    bias_d = nc.dram_tensor("bias", [1, E], f32, kind="ExternalInput").ap()
    idx_d = nc.dram_tensor("idx", [NPC, TOP_K], i32, kind="ExternalOutput").ap()
    w_d = nc.dram_tensor("w", [NPC, TOP_K], f32, kind="ExternalOutput").ap()

    with tile.TileContext(nc) as tc:
        with (
            tc.tile_pool(name="const", bufs=1) as const_pool,
            tc.tile_pool(name="gw", bufs=1) as gw_pool,
            tc.tile_pool(name="x01", bufs=1) as x01_pool,
            tc.tile_pool(name="xh", bufs=2) as xh_pool,
            tc.tile_pool(name="xl", bufs=2) as xl_pool,
            tc.tile_pool(name="plog", bufs=2, space="PSUM") as plog_pool,
            tc.tile_pool(name="junk", bufs=1, space="PSUM") as junk_pool,
            tc.tile_pool(name="work", bufs=2) as work_pool,
            tc.tile_pool(name="outs", bufs=2) as out_pool,
        ):
            # ---- piece tables for tiles 0/1 (separate tiles => exact DMA deps)
            x_pieces = {0: {}, 1: {}}  # (t) -> piece_idx -> (hi_tile, lo_tile)
            x_tiles = {}

            def make_piece(t, pi, chunks):
                w_ = chunks * P
                hi = x01_pool.tile([P, w_], f16, name=f"xh{t}p{pi}")
                lo = x01_pool.tile([P, w_], f16, name=f"xl{t}p{pi}")
                x_pieces[t][pi] = (hi, lo)
                return hi, lo

            def dma_piece(t, pi, chunks):
                hi, lo = x_pieces[t][pi]
                base = t * KC * P + pi * chunks * P
                sl = slice(base, base + chunks * P)
                nc.sync.dma_start(hi[:], xh_d[:, sl])
                nc.sync.dma_start(lo[:], xl_d[:, sl])

            def fetch_x(t):
                x_tiles[t] = (
                    xh_pool.tile([P, KC * P], f16, tag="xh", name=f"xh{t}"),
                    xl_pool.tile([P, KC * P], f16, tag="xl", name=f"xl{t}"),
                )
                sl = slice(t * KC * P, (t + 1) * KC * P)
                nc.sync.dma_start(x_tiles[t][0][:], xh_d[:, sl])
                nc.sync.dma_start(x_tiles[t][1][:], xl_d[:, sl])

            gw_sb = []
            q = GPC * 2 * E
            for i in range(NGP):
                gw_sb.append(gw_pool.tile([P, q], f16, name=f"gw{i}"))
            for t, npc2 in ((0, X0P), (1, X1P)):
                for pi in range(npc2):
                    make_piece(t, pi, KC // npc2)

            # ---- single SP-ring FIFO, hand-interleaved for just-in-time
            # pacing: tile-0 x pieces 1:1 with gw pieces, then tile-1 pieces,
            # then whole tiles 2-3.
            bias_sb = const_pool.tile([1, E], f32, name="biassb")
            nc.sync.dma_start(bias_sb[:], bias_d[:])
            for i in range(NGP):
                dma_piece(0, i, KC // X0P)
                nc.sync.dma_start(gw_sb[i][:], gw_d[:, i * q : (i + 1) * q])
            for pi in range(X1P):
                dma_piece(1, pi, KC // X1P)
            fetch_x(2)
            fetch_x(3)
            gw_v = [g[:].rearrange("p (k e) -> p k e", k=GPC) for g in gw_sb]

            # ---- constants ----
            iota_i = const_pool.tile([P, E], i32)
            nc.gpsimd.iota(iota_i[:], pattern=[[1, E]], base=0, channel_multiplier=0)
            iota_f = const_pool.tile([P, E], f32)
            nc.vector.tensor_copy(iota_f[:], iota_i[:])
            bias_rep = const_pool.tile([P, E], f32)
            nc.gpsimd.partition_broadcast(bias_rep[:], bias_sb[0:1, :])

            # ---- PE warmup: flip the HAM clock gate to 8/8 during the
            # initial DMA wait (needs ~3.4us of sustained PE activity).
            ij = iota_f[:].bitcast(f16)  # [P, 512] garbage-but-finite fp16
            junk = junk_pool.tile([P, 64], f32)
            for _ in range(WARMUP_MM):
                nc.tensor.matmul(junk[:], ij[:, 0:P], ij[:, 0:64], start=True,
                                 stop=True)

            for t in range(TT):
                if 2 <= t <= TT - 3:
                    fetch_x(t + 2)

                def chunk_ap(k):
                    if t in x_pieces:
                        cpp = KC // (X0P if t == 0 else X1P)
                        hi, lo = x_pieces[t][k // cpp]
                        off = (k % cpp) * P
                    else:
                        hi, lo = x_tiles[t]
                        off = k * P
                    return hi[:, off : off + P], lo[:, off : off + P]

                plog = plog_pool.tile([P, E], f32, tag="plog")
                for k in range(KC):
                    pc, sk = divmod(k, GPC)
                    wh = gw_v[pc][:, sk, 0:E]
                    wl = gw_v[pc][:, sk, E : 2 * E]
                    xh_k, xl_k = chunk_ap(k)
                    nc.tensor.matmul(plog[:], xh_k, wh, start=(k == 0), stop=False)
                    nc.tensor.matmul(plog[:], xh_k, wl, start=False, stop=False)
                    nc.tensor.matmul(
                        plog[:], xl_k, wh, start=False, stop=(k == KC - 1)
                    )

                # ---- routing for this token tile ----
                scores = work_pool.tile([P, E], f32, tag="scores")
                nc.scalar.activation(
                    scores[:], plog[:], AF.Sigmoid, scale=1.0 / (SX * SW)
                )

                sfc = work_pool.tile([P, E], f32, tag="sfc")
                nc.vector.tensor_tensor(sfc[:], scores[:], bias_rep[:], op=OP.add)

                # per-group top-8 (need top-2 of each group of 32)
                gtops = work_pool.tile([P, G * 8], f32, tag="gtops")
                for g in range(G):
                    nc.vector.max(
                        gtops[:, g * 8 : (g + 1) * 8],
                        sfc[:, g * EPG : (g + 1) * EPG],
                    )
                gv = gtops[:].rearrange("p (g k) -> p g k", g=G)
                gs = work_pool.tile([P, G], f32, tag="gs")
                nc.vector.tensor_tensor(gs[:], gv[:, :, 0], gv[:, :, 1], op=OP.add)

                # top-4 groups -> mask
                gtop8 = work_pool.tile([P, 8], f32, tag="gtop8")
                nc.vector.max(gtop8[:], gs[:])
                gmask = work_pool.tile([P, G], f32, tag="gmask")
                nc.vector.tensor_scalar(
                    gmask[:], gs[:], gtop8[:, TOPK_GROUP - 1 : TOPK_GROUP], None,
                    op0=OP.is_ge,
                )

                # masked scores
                tmp = work_pool.tile([P, E], f32, tag="tmp")
                for g in range(G):
                    nc.vector.tensor_scalar(
                        tmp[:, g * EPG : (g + 1) * EPG],
                        sfc[:, g * EPG : (g + 1) * EPG],
                        gmask[:, g : g + 1],
                        None,
                        op0=OP.mult,
                    )

                # top-8 values + indices
                vals = work_pool.tile([P, TOP_K], f32, tag="vals")
                nc.vector.max(vals[:], tmp[:])
                idxu = work_pool.tile([P, TOP_K], u32, tag="idxu")
                nc.vector.max_index(idxu[:], vals[:], tmp[:])
                idxf = work_pool.tile([P, TOP_K], f32, tag="idxf")
                nc.vector.tensor_copy(idxf[:], idxu[:])

                # gather raw sigmoid scores at the selected indices
                w8 = out_pool.tile([P, TOP_K], f32, tag="w8")
                scratch = work_pool.tile([P, E], f32, tag="scratch")
                for j in range(TOP_K):
                    nc.vector.scalar_tensor_tensor(
                        scratch[:],
                        iota_f[:],
                        idxf[:, j : j + 1],
                        scores[:],
                        op0=OP.is_equal,
                        op1=OP.mult,
                        accum_out=w8[:, j : j + 1],
                    )

                # normalize + scale (wsum > 0 always: sigmoid outputs)
                wsum = work_pool.tile([P, 1], f32, tag="wsum")
                nc.vector.reduce_sum(wsum[:], w8[:], axis=AX.X)
                wrec = work_pool.tile([P, 1], f32, tag="wrec")
                nc.vector.reciprocal(wrec[:], wsum[:])
                w_out = out_pool.tile([P, TOP_K], f32, tag="wout")
                nc.vector.tensor_scalar(
                    w_out[:], w8[:], wrec[:, 0:1], float(SCALING),
                    op0=OP.mult, op1=OP.mult,
                )
                idx_out = out_pool.tile([P, TOP_K], i32, tag="idxout")
                nc.vector.tensor_copy(idx_out[:], idxu[:])

                # outputs ride the ACT HWDGE ring (never blocks input prefetch)
                nc.scalar.dma_start(idx_d[t * P : (t + 1) * P, :], idx_out[:])
                nc.scalar.dma_start(w_d[t * P : (t + 1) * P, :], w_out[:])

    nc.compile()
    return nc


def _get_nc(**kw):
    key = tuple(sorted(kw.items()))
    if key not in _CACHE:
        _CACHE[key] = build_program(**kw)
    return _CACHE[key]


def _pack_x(a):
    # [8192, 7168] -> [8 cores, 128, TT*KC*128]; per core, partition p holds
    # d = k*128+p, free axis is (token tile t, chunk k, token j) -> x.T chunks.
    return np.ascontiguousarray(
        a.reshape(N_CORES, TT, P, KC, P).transpose(0, 4, 1, 3, 2)
    ).reshape(N_CORES, P, TT * KC * P)


def _prep_inputs(x, gate_w, bias):
    xs = x * np.float32(SX)
    xh = xs.astype(np.float16)
    xl = (xs - xh.astype(np.float32)).astype(np.float16)
    xhp = _pack_x(xh)
    xlp = _pack_x(xl)

    ws = np.ascontiguousarray(gate_w.T) * np.float32(SW)  # [D, E]
    wh = ws.astype(np.float16)
    wl = (ws - wh.astype(np.float32)).astype(np.float16)
    gw2 = np.concatenate(
        [wh.reshape(KC, P, E), wl.reshape(KC, P, E)], axis=2
    )  # [KC, P, 2E]
    gw2 = np.ascontiguousarray(gw2.transpose(1, 0, 2)).reshape(P, KC * 2 * E)
    bias2d = np.ascontiguousarray(bias.reshape(1, E))
    return xhp, xlp, gw2, bias2d


def _run(x, gate_w, bias, trace=False, **build_kw):
    from concourse.bass_utils import run_bass_kernel_spmd

    x = np.ascontiguousarray(np.asarray(x, dtype=np.float32))
    gate_w = np.ascontiguousarray(np.asarray(gate_w, dtype=np.float32))
    bias = np.ascontiguousarray(np.asarray(bias, dtype=np.float32))
    nc = _get_nc(**build_kw)
    xhp, xlp, gw2, bias2d = _prep_inputs(x, gate_w, bias)
    in_maps = [
        {"xh": xhp[c], "xl": xlp[c], "gw2": gw2, "bias": bias2d}
        for c in range(N_CORES)
    ]
    res = run_bass_kernel_spmd(nc, in_maps, core_ids=list(range(N_CORES)), trace=trace)
    idx = np.concatenate([res.results[c]["idx"] for c in range(N_CORES)], axis=0)
    w = np.concatenate([res.results[c]["w"] for c in range(N_CORES)], axis=0)
    return (idx.astype(np.int32), w.astype(np.float32)), res


def kernel(x, gate_w, bias):
    (idx, w), _ = _run(x, gate_w, bias)
    return idx, w


# revision 10
# speedup vs baseline: 1.6750x; 1.0130x over previous
"""DeepSeek-V3 token-choice top-k router on 8 Trainium2 NeuronCores.

Strategy (per core, data-parallel over tokens; 1024 tokens/core):
  - Host: x and gate_w.T are scaled by 4096 (exact power of 2) and split into
    fp16 hi/lo pairs (hi = rn(a), lo = rn(a - hi); 22+ mantissa bits combined,
    scaling keeps everything out of the fp16 subnormal range). x is also
    pre-transposed to d-major [128d, token] chunk layout so the device does
    ZERO transposes and zero precision-split work.
  - Device per 128-token tile: 56 contraction chunks x 3 accumulating fp16
    matmuls (xh@wh + xh@wl + xl@wh; dropped xl@wl term is ~2^-22 relative)
    into a [128, 256] PSUM logits tile. fp16 streams the PE at 1 col/cycle
    (same as f32r) but halves gate DMA and speeds LDWEIGHTS (FWL-eligible).
  - All input DMA goes through the single SP HWDGE ring in a hand-interleaved
    order (x0/x1 in pieces between gw pieces) so tile-0/1 matmuls start ~6us
    in and stay just-in-time fed; output DMA rides the ACT ring.
  - ~70 tiny warmup matmuls run during the initial DMA wait to flip the PE
    HAM clock gate to 2.4 GHz before real work arrives.
  - ACT: sigmoid(logits * 2^-24) PSUM->SBUF (scale undone for free).
  - DVE: bias add, hardware top-8 (`max`/`max_index`) for group top-2 sums,
    top-4 group threshold, masked top-8, one-hot score gathers, normalization.
"""

import numpy as np

N = 8192
D = 7168
E = 256
G = 8
EPG = E // G  # 32
TOPK_GROUP = 4
TOP_K = 8
SCALING = 2.5
N_CORES = 8
NPC = N // N_CORES  # 1024 tokens per core
P = 128
KC = D // P  # 56 contraction chunks
TT = NPC // P  # 8 token tiles per core
GPC = 7  # gate-weight chunks per DMA piece
NGP = KC // GPC  # 8 pieces
SX = 4096.0  # x scale (2^12)
SW = 4096.0  # w scale (2^12)
X0P = 8  # tile-0/1 x DMA pieces (7 chunks each)
X1P = 8
WARMUP_MM = 48

_CACHE = {}


def build_program():
    import concourse.bacc as bacc
    import concourse.mybir as mybir
    from concourse import tile

    nc = bacc.Bacc(
        "TRN2",
        target_bir_lowering=False,
        debug=False,
        enable_asserts=True,
        num_devices=N_CORES,
    )
    f16 = mybir.dt.float16
    f32 = mybir.dt.float32
    i32 = mybir.dt.int32
    u32 = mybir.dt.uint32
    AF = mybir.ActivationFunctionType
    OP = mybir.AluOpType
    AX = mybir.AxisListType

    xh_d = nc.dram_tensor("xh", [P, TT * KC * P], f16, kind="ExternalInput").ap()
    xl_d = nc.dram_tensor("xl", [P, TT * KC * P], f16, kind="ExternalInput").ap()
    gw_d = nc.dram_tensor("gw2", [P, KC * 2 * E], f16, kind="ExternalInput").ap()
    bias_d = nc.dram_tensor("bias", [1, E], f32, kind="ExternalInput").ap()
    idx_d = nc.dram_tensor("idx", [NPC, TOP_K], i32, kind="ExternalOutput").ap()
    w_d = nc.dram_tensor("w", [NPC, TOP_K], f32, kind="ExternalOutput").ap()

    with tile.TileContext(nc) as tc:
        with (
            tc.tile_pool(name="const", bufs=1) as const_pool,
            tc.tile_pool(name="gw", bufs=1) as gw_pool,
            tc.tile_pool(name="x01", bufs=1) as x01_pool,
            tc.tile_pool(name="xh", bufs=2) as xh_pool,
            tc.tile_pool(name="xl", bufs=2) as xl_pool,
            tc.tile_pool(name="plog", bufs=3, space="PSUM") as plog_pool,
            tc.tile_pool(name="junk", bufs=1, space="PSUM") as junk_pool,
            tc.tile_pool(name="work", bufs=2) as work_pool,
            tc.tile_pool(name="outs", bufs=2) as out_pool,
        ):
            # ---- piece tables for tiles 0/1 (separate tiles => exact DMA deps)
            x_pieces = {0: {}, 1: {}}  # (t) -> piece_idx -> (hi_tile, lo_tile)
            x_tiles = {}

            def make_piece(t, pi, chunks):
                w_ = chunks * P
                hi = x01_pool.tile([P, w_], f16, name=f"xh{t}p{pi}")
                lo = x01_pool.tile([P, w_], f16, name=f"xl{t}p{pi}")
                x_pieces[t][pi] = (hi, lo)
                return hi, lo

            def dma_piece(t, pi, chunks):
                hi, lo = x_pieces[t][pi]
                base = t * KC * P + pi * chunks * P
                sl = slice(base, base + chunks * P)
                nc.sync.dma_start(hi[:], xh_d[:, sl])
                nc.sync.dma_start(lo[:], xl_d[:, sl])

            def fetch_x(t):
                x_tiles[t] = (
                    xh_pool.tile([P, KC * P], f16, tag="xh", name=f"xh{t}"),
                    xl_pool.tile([P, KC * P], f16, tag="xl", name=f"xl{t}"),
                )
                sl = slice(t * KC * P, (t + 1) * KC * P)
                nc.sync.dma_start(x_tiles[t][0][:], xh_d[:, sl])
                nc.sync.dma_start(x_tiles[t][1][:], xl_d[:, sl])

            gw_sb = []
            q = GPC * 2 * E
            for i in range(NGP):
                gw_sb.append(gw_pool.tile([P, q], f16, name=f"gw{i}"))
            for t, npc2 in ((0, X0P), (1, X1P)):
                for pi in range(npc2):
                    make_piece(t, pi, KC // npc2)

            # ---- single SP-ring FIFO, hand-interleaved for just-in-time
            # pacing: per piece group i deliver [x0 piece i, gw piece i,
            # x1 piece i] — each group unlocks 14 chunks of ping-ponged
            # tile-0/1 matmul work, keeping PE fed while gw streams in.
            bias_sb = const_pool.tile([1, E], f32, name="biassb")
            nc.sync.dma_start(bias_sb[:], bias_d[:])
            for i in range(NGP):
                dma_piece(0, i, KC // X0P)
                nc.sync.dma_start(gw_sb[i][:], gw_d[:, i * q : (i + 1) * q])
                dma_piece(1, i, KC // X1P)
            fetch_x(2)
            fetch_x(3)
            gw_v = [g[:].rearrange("p (k e) -> p k e", k=GPC) for g in gw_sb]

            # ---- constants ----
            iota_i = const_pool.tile([P, E], i32)
            nc.gpsimd.iota(iota_i[:], pattern=[[1, E]], base=0, channel_multiplier=0)
            iota_f = const_pool.tile([P, E], f32)
            nc.vector.tensor_copy(iota_f[:], iota_i[:])
            bias_rep = const_pool.tile([P, E], f32)
            nc.gpsimd.partition_broadcast(bias_rep[:], bias_sb[0:1, :])

            # ---- PE warmup: flip the HAM clock gate to 8/8 during the
            # initial DMA wait (needs ~3.4us of sustained PE activity).
            ij = iota_f[:].bitcast(f16)  # [P, 512] garbage-but-finite fp16
            junk = junk_pool.tile([P, 64], f32)
            for _ in range(WARMUP_MM):
                nc.tensor.matmul(junk[:], ij[:, 0:P], ij[:, 0:64], start=True,
                                 stop=True)

            def chunk_ap(t, k):
                if t in x_pieces:
                    cpp = KC // (X0P if t == 0 else X1P)
                    hi, lo = x_pieces[t][k // cpp]
                    off = (k % cpp) * P
                else:
                    hi, lo = x_tiles[t]
                    off = k * P
                return hi[:, off : off + P], lo[:, off : off + P]

            def mm_chunks(t, plog, k0, k1):
                for k in range(k0, k1):
                    pc, sk = divmod(k, GPC)
                    wh = gw_v[pc][:, sk, 0:E]
                    wl = gw_v[pc][:, sk, E : 2 * E]
                    xh_k, xl_k = chunk_ap(t, k)
                    nc.tensor.matmul(plog[:], xh_k, wh, start=(k == 0), stop=False)
                    nc.tensor.matmul(plog[:], xh_k, wl, start=False, stop=False)
                    nc.tensor.matmul(
                        plog[:], xl_k, wh, start=False, stop=(k == KC - 1)
                    )

            def routing(t, plog):
                # ---- routing for this token tile ----
                scores = work_pool.tile([P, E], f32, tag="scores")
                nc.scalar.activation(
                    scores[:], plog[:], AF.Sigmoid, scale=1.0 / (SX * SW)
                )

                sfc = work_pool.tile([P, E], f32, tag="sfc")
                nc.vector.tensor_tensor(sfc[:], scores[:], bias_rep[:], op=OP.add)

                # per-group top-8 (need top-2 of each group of 32)
                gtops = work_pool.tile([P, G * 8], f32, tag="gtops")
                for g in range(G):
                    nc.vector.max(
                        gtops[:, g * 8 : (g + 1) * 8],
                        sfc[:, g * EPG : (g + 1) * EPG],
                    )
                gv = gtops[:].rearrange("p (g k) -> p g k", g=G)
                gs = work_pool.tile([P, G], f32, tag="gs")
                nc.vector.tensor_tensor(gs[:], gv[:, :, 0], gv[:, :, 1], op=OP.add)

                # top-4 groups -> mask
                gtop8 = work_pool.tile([P, 8], f32, tag="gtop8")
                nc.vector.max(gtop8[:], gs[:])
                gmask = work_pool.tile([P, G], f32, tag="gmask")
                nc.vector.tensor_scalar(
                    gmask[:], gs[:], gtop8[:, TOPK_GROUP - 1 : TOPK_GROUP], None,
                    op0=OP.is_ge,
                )

                # masked scores
                tmp = work_pool.tile([P, E], f32, tag="tmp")
                for g in range(G):
                    nc.vector.tensor_scalar(
                        tmp[:, g * EPG : (g + 1) * EPG],
                        sfc[:, g * EPG : (g + 1) * EPG],
                        gmask[:, g : g + 1],
                        None,
                        op0=OP.mult,
                    )

                # top-8 values + indices
                vals = work_pool.tile([P, TOP_K], f32, tag="vals")
                nc.vector.max(vals[:], tmp[:])
                idxu = work_pool.tile([P, TOP_K], u32, tag="idxu")
                nc.vector.max_index(idxu[:], vals[:], tmp[:])
                idxf = work_pool.tile([P, TOP_K], f32, tag="idxf")
                nc.vector.tensor_copy(idxf[:], idxu[:])

                # gather raw sigmoid scores at the selected indices
                w8 = out_pool.tile([P, TOP_K], f32, tag="w8")
                scratch = work_pool.tile([P, E], f32, tag="scratch")
                for j in range(TOP_K):
                    nc.vector.scalar_tensor_tensor(
                        scratch[:],
                        iota_f[:],
                        idxf[:, j : j + 1],
                        scores[:],
                        op0=OP.is_equal,
                        op1=OP.mult,
                        accum_out=w8[:, j : j + 1],
                    )

                # normalize + scale (wsum > 0 always: sigmoid outputs)
                wsum = work_pool.tile([P, 1], f32, tag="wsum")
                nc.vector.reduce_sum(wsum[:], w8[:], axis=AX.X)
                wrec = work_pool.tile([P, 1], f32, tag="wrec")
                nc.vector.reciprocal(wrec[:], wsum[:])
                w_out = out_pool.tile([P, TOP_K], f32, tag="wout")
                nc.vector.tensor_scalar(
                    w_out[:], w8[:], wrec[:, 0:1], float(SCALING),
                    op0=OP.mult, op1=OP.mult,
                )
                idx_out = out_pool.tile([P, TOP_K], i32, tag="idxout")
                nc.vector.tensor_copy(idx_out[:], idxu[:])

                # outputs ride the ACT HWDGE ring (never blocks input prefetch)
                nc.scalar.dma_start(idx_d[t * P : (t + 1) * P, :], idx_out[:])
                nc.scalar.dma_start(w_d[t * P : (t + 1) * P, :], w_out[:])

            # ---- tiles 0/1: ping-pong at piece granularity so PE work
            # tracks DMA arrival during the gw-streaming window.
            plogs = {
                t: plog_pool.tile([P, E], f32, tag="plog", name=f"plog{t}")
                for t in (0, 1)
            }
            for i in range(NGP):
                mm_chunks(0, plogs[0], i * GPC, (i + 1) * GPC)
                mm_chunks(1, plogs[1], i * GPC, (i + 1) * GPC)
            routing(0, plogs[0])
            routing(1, plogs[1])

            # ---- tiles 2..7: straight pipeline, prefetch distance 2
            for t in range(2, TT):
                if t + 2 < TT:
                    fetch_x(t + 2)
                plog = plog_pool.tile([P, E], f32, tag="plog", name=f"plog{t}")
                mm_chunks(t, plog, 0, KC)
                routing(t, plog)

    nc.compile()
    return nc


def _get_nc(**kw):
    key = tuple(sorted(kw.items()))
    if key not in _CACHE:
        _CACHE[key] = build_program(**kw)
    return _CACHE[key]


def _pack_x(a):
    # [8192, 7168] -> [8 cores, 128, TT*KC*128]; per core, partition p holds
    # d = k*128+p, free axis is (token tile t, chunk k, token j) -> x.T chunks.
    return np.ascontiguousarray(
        a.reshape(N_CORES, TT, P, KC, P).transpose(0, 4, 1, 3, 2)
    ).reshape(N_CORES, P, TT * KC * P)


def _prep_inputs(x, gate_w, bias):
    xs = x * np.float32(SX)
    xh = xs.astype(np.float16)
    xl = (xs - xh.astype(np.float32)).astype(np.float16)
    xhp = _pack_x(xh)
    xlp = _pack_x(xl)

    ws = np.ascontiguousarray(gate_w.T) * np.float32(SW)  # [D, E]
    wh = ws.astype(np.float16)
    wl = (ws - wh.astype(np.float32)).astype(np.float16)
    gw2 = np.concatenate(
        [wh.reshape(KC, P, E), wl.reshape(KC, P, E)], axis=2
    )  # [KC, P, 2E]
    gw2 = np.ascontiguousarray(gw2.transpose(1, 0, 2)).reshape(P, KC * 2 * E)
    bias2d = np.ascontiguousarray(bias.reshape(1, E))
    return xhp, xlp, gw2, bias2d


def _run(x, gate_w, bias, trace=False, **build_kw):
    from concourse.bass_utils import run_bass_kernel_spmd

    x = np.ascontiguousarray(np.asarray(x, dtype=np.float32))
    gate_w = np.ascontiguousarray(np.asarray(gate_w, dtype=np.float32))
    bias = np.ascontiguousarray(np.asarray(bias, dtype=np.float32))
    nc = _get_nc(**build_kw)
    xhp, xlp, gw2, bias2d = _prep_inputs(x, gate_w, bias)
    in_maps = [
        {"xh": xhp[c], "xl": xlp[c], "gw2": gw2, "bias": bias2d}
        for c in range(N_CORES)
    ]
    res = run_bass_kernel_spmd(nc, in_maps, core_ids=list(range(N_CORES)), trace=trace)
    idx = np.concatenate([res.results[c]["idx"] for c in range(N_CORES)], axis=0)
    w = np.concatenate([res.results[c]["w"] for c in range(N_CORES)], axis=0)
    return (idx.astype(np.int32), w.astype(np.float32)), res


def kernel(x, gate_w, bias):
    (idx, w), _ = _run(x, gate_w, bias)
    return idx, w


# revision 11
# speedup vs baseline: 1.7507x; 1.0451x over previous
"""DeepSeek-V3 token-choice top-k router on 8 Trainium2 NeuronCores.

Strategy (per core, data-parallel over tokens; 1024 tokens/core):
  - Host: x and gate_w.T are scaled by 4096 (exact power of 2) and split into
    fp16 hi/lo pairs (hi = rn(a), lo = rn(a - hi); 22+ mantissa bits combined,
    scaling keeps everything out of the fp16 subnormal range). x is also
    pre-transposed to d-major [128d, token] chunk layout so the device does
    ZERO transposes and zero precision-split work; hi/lo are packed adjacent
    per 7-chunk piece so one DMA descriptor fetches both.
  - Device per 128-token tile: 56 contraction chunks x 3 accumulating fp16
    matmuls (xh@wh + xh@wl + xl@wh; dropped xl@wl term is ~2^-22 relative)
    into a [128, 256] PSUM logits tile. fp16 streams the PE at 1 col/cycle
    (same as f32r) but halves gate DMA and speeds LDWEIGHTS.
  - x streams through a 12-deep piece pool on the SP HWDGE ring; gate weight
    pieces ride the ACT ring so both FIFOs pace independently. The first
    three tiles are ping-ponged at piece granularity (21 chunks of PE work
    per piece group) so the PE stays fed while the gate weight streams in;
    ~40 warmup matmuls flip the PE HAM clock gate to 2.4 GHz first.
  - ACT: sigmoid(logits * 2^-24) PSUM->SBUF (scale undone for free).
  - DVE: bias add, hardware top-8 (`max`/`max_index`) for group top-2 sums,
    top-4 group threshold, masked top-8, one-hot score gathers, normalization.
"""

import numpy as np

N = 8192
D = 7168
E = 256
G = 8
EPG = E // G  # 32
TOPK_GROUP = 4
TOP_K = 8
SCALING = 2.5
N_CORES = 8
NPC = N // N_CORES  # 1024 tokens per core
P = 128
KC = D // P  # 56 contraction chunks
TT = NPC // P  # 8 token tiles per core
GPC = 7  # chunks per piece (both x and gate-weight DMA granularity)
NGP = KC // GPC  # 8 pieces per tile
PW = 2 * GPC * P  # 1792: piece width in fp16 elems (hi 896 | lo 896)
PINGPONG = 3  # tiles interleaved at startup
WARMUP_MM = 40
SX = 4096.0  # x scale (2^12)
SW = 4096.0  # w scale (2^12)

_CACHE = {}


def build_program():
    import concourse.bacc as bacc
    import concourse.mybir as mybir
    from concourse import tile

    nc = bacc.Bacc(
        "TRN2",
        target_bir_lowering=False,
        debug=False,
        enable_asserts=True,
        num_devices=N_CORES,
    )
    f16 = mybir.dt.float16
    f32 = mybir.dt.float32
    i32 = mybir.dt.int32
    u32 = mybir.dt.uint32
    AF = mybir.ActivationFunctionType
    OP = mybir.AluOpType
    AX = mybir.AxisListType

    x_d = nc.dram_tensor("x2", [P, TT * NGP * PW], f16, kind="ExternalInput").ap()
    gw_d = nc.dram_tensor("gw2", [P, KC * 2 * E], f16, kind="ExternalInput").ap()
    bias_d = nc.dram_tensor("bias", [1, E], f32, kind="ExternalInput").ap()
    idx_d = nc.dram_tensor("idx", [NPC, TOP_K], i32, kind="ExternalOutput").ap()
    w_d = nc.dram_tensor("w", [NPC, TOP_K], f32, kind="ExternalOutput").ap()

    with tile.TileContext(nc) as tc:
        with (
            tc.tile_pool(name="const", bufs=1) as const_pool,
            tc.tile_pool(name="gw", bufs=1) as gw_pool,
            tc.tile_pool(name="xp", bufs=12) as x_pool,
            tc.tile_pool(name="plog", bufs=4, space="PSUM") as plog_pool,
            tc.tile_pool(name="junk", bufs=1, space="PSUM") as junk_pool,
            tc.tile_pool(name="work", bufs=2) as work_pool,
            tc.tile_pool(name="outs", bufs=2) as out_pool,
        ):
            # ---- gate weight pieces on the ACT HWDGE ring ----
            bias_sb = const_pool.tile([1, E], f32, name="biassb")
            nc.scalar.dma_start(bias_sb[:], bias_d[:])
            gw_sb = []
            q = GPC * 2 * E
            for i in range(NGP):
                gw_sb.append(gw_pool.tile([P, q], f16, name=f"gw{i}"))
                nc.scalar.dma_start(gw_sb[i][:], gw_d[:, i * q : (i + 1) * q])
            gw_v = [g[:].rearrange("p (k e) -> p k e", k=GPC) for g in gw_sb]

            # ---- x pieces on the SP ring, in exact consumption order; the
            # 12-deep pool + FIFO self-pace the prefetch.
            pieces = {}

            def fetch_piece(t, pi):
                pc = x_pool.tile([P, PW], f16, tag="xp", name=f"x{t}p{pi}")
                pieces[(t, pi)] = pc
                base = (t * NGP + pi) * PW
                nc.sync.dma_start(pc[:], x_d[:, base : base + PW])

            for i in range(NGP):
                for t in range(PINGPONG):
                    fetch_piece(t, i)
            for t in range(PINGPONG, TT):
                for i in range(NGP):
                    fetch_piece(t, i)

            # ---- constants ----
            iota_i = const_pool.tile([P, E], i32)
            nc.gpsimd.iota(iota_i[:], pattern=[[1, E]], base=0, channel_multiplier=0)
            iota_f = const_pool.tile([P, E], f32)
            nc.vector.tensor_copy(iota_f[:], iota_i[:])
            bias_rep = const_pool.tile([P, E], f32)
            nc.gpsimd.partition_broadcast(bias_rep[:], bias_sb[0:1, :])

            # ---- PE warmup: flip the HAM clock gate to 8/8 during the
            # initial DMA wait (keeps the first real matmuls at 2.4 GHz).
            ij = iota_f[:].bitcast(f16)  # [P, 512] garbage-but-finite fp16
            junk = junk_pool.tile([P, 64], f32)
            for _ in range(WARMUP_MM):
                nc.tensor.matmul(junk[:], ij[:, 0:P], ij[:, 0:64], start=True,
                                 stop=True)

            def chunk_ap(t, k):
                pc = pieces[(t, k // GPC)]
                off = (k % GPC) * P
                return pc[:, off : off + P], pc[:, GPC * P + off : GPC * P + off + P]

            def mm_chunks(t, plog, k0, k1):
                for k in range(k0, k1):
                    pc, sk = divmod(k, GPC)
                    wh = gw_v[pc][:, sk, 0:E]
                    wl = gw_v[pc][:, sk, E : 2 * E]
                    xh_k, xl_k = chunk_ap(t, k)
                    nc.tensor.matmul(plog[:], xh_k, wh, start=(k == 0), stop=False)
                    nc.tensor.matmul(plog[:], xh_k, wl, start=False, stop=False)
                    nc.tensor.matmul(
                        plog[:], xl_k, wh, start=False, stop=(k == KC - 1)
                    )

            def routing(t, plog):
                # ---- routing for this token tile ----
                scores = work_pool.tile([P, E], f32, tag="scores")
                nc.scalar.activation(
                    scores[:], plog[:], AF.Sigmoid, scale=1.0 / (SX * SW)
                )

                sfc = work_pool.tile([P, E], f32, tag="sfc")
                nc.vector.tensor_tensor(sfc[:], scores[:], bias_rep[:], op=OP.add)

                # per-group top-8 (need top-2 of each group of 32)
                gtops = work_pool.tile([P, G * 8], f32, tag="gtops")
                for g in range(G):
                    nc.vector.max(
                        gtops[:, g * 8 : (g + 1) * 8],
                        sfc[:, g * EPG : (g + 1) * EPG],
                    )
                gv = gtops[:].rearrange("p (g k) -> p g k", g=G)
                gs = work_pool.tile([P, G], f32, tag="gs")
                nc.vector.tensor_tensor(gs[:], gv[:, :, 0], gv[:, :, 1], op=OP.add)

                # top-4 groups -> mask
                gtop8 = work_pool.tile([P, 8], f32, tag="gtop8")
                nc.vector.max(gtop8[:], gs[:])
                gmask = work_pool.tile([P, G], f32, tag="gmask")
                nc.vector.tensor_scalar(
                    gmask[:], gs[:], gtop8[:, TOPK_GROUP - 1 : TOPK_GROUP], None,
                    op0=OP.is_ge,
                )

                # masked scores
                tmp = work_pool.tile([P, E], f32, tag="tmp")
                for g in range(G):
                    nc.vector.tensor_scalar(
                        tmp[:, g * EPG : (g + 1) * EPG],
                        sfc[:, g * EPG : (g + 1) * EPG],
                        gmask[:, g : g + 1],
                        None,
                        op0=OP.mult,
                    )

                # top-8 values + indices
                vals = work_pool.tile([P, TOP_K], f32, tag="vals")
                nc.vector.max(vals[:], tmp[:])
                idxu = work_pool.tile([P, TOP_K], u32, tag="idxu")
                nc.vector.max_index(idxu[:], vals[:], tmp[:])
                idxf = work_pool.tile([P, TOP_K], f32, tag="idxf")
                nc.vector.tensor_copy(idxf[:], idxu[:])

                # gather raw sigmoid scores at the selected indices
                w8 = out_pool.tile([P, TOP_K], f32, tag="w8")
                scratch = work_pool.tile([P, E], f32, tag="scratch")
                for j in range(TOP_K):
                    nc.vector.scalar_tensor_tensor(
                        scratch[:],
                        iota_f[:],
                        idxf[:, j : j + 1],
                        scores[:],
                        op0=OP.is_equal,
                        op1=OP.mult,
                        accum_out=w8[:, j : j + 1],
                    )

                # normalize + scale (wsum > 0 always: sigmoid outputs)
                wsum = work_pool.tile([P, 1], f32, tag="wsum")
                nc.vector.reduce_sum(wsum[:], w8[:], axis=AX.X)
                wrec = work_pool.tile([P, 1], f32, tag="wrec")
                nc.vector.reciprocal(wrec[:], wsum[:])
                w_out = out_pool.tile([P, TOP_K], f32, tag="wout")
                nc.vector.tensor_scalar(
                    w_out[:], w8[:], wrec[:, 0:1], float(SCALING),
                    op0=OP.mult, op1=OP.mult,
                )
                idx_out = out_pool.tile([P, TOP_K], i32, tag="idxout")
                nc.vector.tensor_copy(idx_out[:], idxu[:])

                # outputs ride the ACT HWDGE ring (never blocks input prefetch)
                nc.scalar.dma_start(idx_d[t * P : (t + 1) * P, :], idx_out[:])
                nc.scalar.dma_start(w_d[t * P : (t + 1) * P, :], w_out[:])

            # ---- tiles 0..2: ping-pong at piece granularity so PE work
            # tracks DMA arrival while the gate weight streams in.
            plogs = {
                t: plog_pool.tile([P, E], f32, tag="plog", name=f"plog{t}")
                for t in range(PINGPONG)
            }
            for i in range(NGP):
                for t in range(PINGPONG):
                    mm_chunks(t, plogs[t], i * GPC, (i + 1) * GPC)
            for t in range(PINGPONG):
                routing(t, plogs[t])

            # ---- tiles 3..7: straight pipeline (pieces prefetched by pool)
            for t in range(PINGPONG, TT):
                plog = plog_pool.tile([P, E], f32, tag="plog", name=f"plog{t}")
                mm_chunks(t, plog, 0, KC)
                routing(t, plog)

    nc.compile()
    return nc


def _get_nc(**kw):
    key = tuple(sorted(kw.items()))
    if key not in _CACHE:
        _CACHE[key] = build_program(**kw)
    return _CACHE[key]


def _pack_x(xh, xl):
    # [8192, 7168] fp16 hi/lo -> [8 cores, 128, TT*NGP*1792]; per core,
    # partition p holds d = k*128+p; pieces of 7 chunks with hi|lo adjacent.
    def five(a):
        # [c, t, j, k, p] -> [c, p, t, k, j] -> split k into (pi, ks)
        b = a.reshape(N_CORES, TT, P, KC, P).transpose(0, 4, 1, 3, 2)
        return b.reshape(N_CORES, P, TT, NGP, GPC, P)

    comb = np.stack([five(xh), five(xl)], axis=4)  # [c,p,t,pi,{h,l},ks,j]
    return np.ascontiguousarray(comb).reshape(N_CORES, P, TT * NGP * PW)


def _prep_inputs(x, gate_w, bias):
    xs = x * np.float32(SX)
    xh = xs.astype(np.float16)
    xl = (xs - xh.astype(np.float32)).astype(np.float16)
    xp = _pack_x(xh, xl)

    ws = np.ascontiguousarray(gate_w.T) * np.float32(SW)  # [D, E]
    wh = ws.astype(np.float16)
    wl = (ws - wh.astype(np.float32)).astype(np.float16)
    gw2 = np.concatenate(
        [wh.reshape(KC, P, E), wl.reshape(KC, P, E)], axis=2
    )  # [KC, P, 2E]
    gw2 = np.ascontiguousarray(gw2.transpose(1, 0, 2)).reshape(P, KC * 2 * E)
    bias2d = np.ascontiguousarray(bias.reshape(1, E))
    return xp, gw2, bias2d


def _run(x, gate_w, bias, trace=False, **build_kw):
    from concourse.bass_utils import run_bass_kernel_spmd

    x = np.ascontiguousarray(np.asarray(x, dtype=np.float32))
    gate_w = np.ascontiguousarray(np.asarray(gate_w, dtype=np.float32))
    bias = np.ascontiguousarray(np.asarray(bias, dtype=np.float32))
    nc = _get_nc(**build_kw)
    xp, gw2, bias2d = _prep_inputs(x, gate_w, bias)
    in_maps = [
        {"x2": xp[c], "gw2": gw2, "bias": bias2d} for c in range(N_CORES)
    ]
    res = run_bass_kernel_spmd(nc, in_maps, core_ids=list(range(N_CORES)), trace=trace)
    idx = np.concatenate([res.results[c]["idx"] for c in range(N_CORES)], axis=0)
    w = np.concatenate([res.results[c]["w"] for c in range(N_CORES)], axis=0)
    return (idx.astype(np.int32), w.astype(np.float32)), res


def kernel(x, gate_w, bias):
    (idx, w), _ = _run(x, gate_w, bias)
    return idx, w
